# revision 10
# baseline (speedup 1.0000x reference)
"""Trainium2 Bass kernel for nn_AdditiveRecursiveNN (depth-13 binary tree of
64x64 matmuls with per-node weights gathered from a 50000x4096 table).

Sharding: data-parallel over the 16 independent depth-9 subtrees rooted at
heap nodes 15..30 -- TWO subtrees per NeuronCore, one per SBUF partition
half. The host packs each core's 1022 node weight matrices into a dense,
matmul-ready stream (pre-transposed W^T strips); each core runs both of its
subtrees bottom-up: h = relu(W @ (h_l + h_r) + b). The top 4 levels (15
nodes) are combined on the host.

Layout: "L" subtree lives in partitions 0:64, "R" in 64:128, node i of a
level at 64-column slot i. Everything is partition-aligned:
 - the pairwise child add is ONE full-width strided DVE op per level,
 - L/R matmuls use DIAGONAL PE quadrants (tile_position (0,0)/(64,64)),
   which are column-disjoint and run concurrently (same-column independent
   quadrant pairs like (0,0)+(64,0) fault the PE),
 - per-node bias is accumulated into PSUM first via K=1 ones-outer-product
   matmuls, so the PSUM drain is a single fused relu+bias on ScalarE.
"""
import sys
sys.path.insert(0, '/opt/trn_rl_repo')

import numpy as np

E = 64
D = 13
N_NODES = 2 ** D - 1          # 8191
NCORES = 8
HT_D = 9                      # half-tree depth: levels 0..8
HT_LEAF = 2 ** (HT_D - 1)     # 256 leaves per half-tree
HT_INT = HT_LEAF - 1          # 255 internal nodes per half-tree
WT_SLOTS = HT_INT             # 255 (shared by wt and bi streams)
LF_SLOTS = HT_LEAF            # 256

_CACHE = {}
BIAS_MODE = "mm"


def _build_nc():
    import concourse.bacc as bacc
    import concourse.tile as tile
    import concourse.mybir as mybir

    f32 = mybir.dt.float32
    nc = bacc.Bacc(None, target_bir_lowering=False)

    wt = nc.dram_tensor("wt", [128, WT_SLOTS * E], f32, kind="ExternalInput")
    lf = nc.dram_tensor("lf", [128, LF_SLOTS * E], f32, kind="ExternalInput")
    bi = nc.dram_tensor("bi", [2, WT_SLOTS * E], f32, kind="ExternalInput")
    out = nc.dram_tensor("out", [128, E], f32, kind="ExternalOutput")

    with tile.TileContext(nc) as tc:
        with (
            tc.tile_pool(name="leaf", bufs=1) as pool_leaf,
            tc.tile_pool(name="h", bufs=2) as pool_h,
            tc.tile_pool(name="s", bufs=1) as pool_s,
            tc.tile_pool(name="wtp", bufs=2) as pool_wt,
            tc.tile_pool(name="btp", bufs=4) as pool_bt,
            tc.tile_pool(name="cst", bufs=1) as pool_c,
            tc.tile_pool(name="ps", bufs=8, space="PSUM") as pool_ps,
        ):
            ones = pool_c.tile([128, E], f32)
            nc.gpsimd.memset(ones[:], 1.0)

            # ---- leaves: stream in, relu in place -> h8 ----
            h_prev = pool_leaf.tile([128, LF_SLOTS * E], f32)
            LCH = 32  # slots per chunk
            for t in range(0, LF_SLOTS, LCH):
                sl = slice(t * E, (t + LCH) * E)
                nc.sync.dma_start(h_prev[:, sl], lf[:, sl])
                nc.vector.tensor_scalar_max(h_prev[:, sl], h_prev[:, sl], 0.0)

            woff = 0  # slot offset into wt/bi streams
            for lvl in range(HT_D - 2, -1, -1):
                n = 2 ** lvl  # nodes per half-tree this level
                # ---- s_i = h_2i + h_2i+1 : one strided DVE op ----
                s = pool_s.tile([128, n * E], f32, tag="s")
                pairs = h_prev[:, 0:2 * n * E].rearrange(
                    "p (t c) -> p t c", c=2 * E)
                nc.vector.tensor_add(
                    s[:].rearrange("p (t m) -> p t m", m=E),
                    pairs[:, :, 0:E], pairs[:, :, E:2 * E])

                h_new = pool_h.tile([128, n * E], f32, tag="h")

                CH = 32  # slots per weight chunk
                for t0 in range(0, n, CH):
                    tn = min(CH, n - t0)
                    w = pool_wt.tile([128, CH * E], f32, tag="wt")
                    nc.sync.dma_start(
                        w[:, 0:tn * E],
                        wt[:, (woff + t0) * E:(woff + t0 + tn) * E])
                    for i0 in range(t0, t0 + tn, 8):
                        g = min(8, t0 + tn - i0)
                        ps = pool_ps.tile([128, 8 * E], f32, tag="ps")
                        first = (BIAS_MODE != "mm")
                        if BIAS_MODE == "mm":
                            bt = pool_bt.tile([128, 8 * E], f32, tag="bt")
                            boff = (woff + i0) * E
                            nc.sync.dma_start(
                                bt[0:1, 0:g * E], bi[0:1, boff:boff + g * E])
                            nc.sync.dma_start(
                                bt[E:E + 1, 0:g * E],
                                bi[1:2, boff:boff + g * E])
                            nc.tensor.matmul(
                                out=ps[0:E, 0:g * E], lhsT=ones[0:1, :],
                                rhs=bt[0:1, 0:g * E], start=True, stop=False,
                                tile_position=(0, 0), skip_group_check=True)
                            nc.tensor.matmul(
                                out=ps[E:128, 0:g * E], lhsT=ones[E:E + 1, :],
                                rhs=bt[E:E + 1, 0:g * E], start=True,
                                stop=False, tile_position=(E, E),
                                skip_group_check=True)
                        for i in range(i0, i0 + g):
                            psl = slice((i - i0) * E, (i - i0 + 1) * E)
                            wsl = slice((i - t0) * E, (i - t0 + 1) * E)
                            ssl = slice(i * E, (i + 1) * E)
                            nc.tensor.matmul(
                                out=ps[0:E, psl], lhsT=w[0:E, wsl],
                                rhs=s[0:E, ssl], start=first, stop=True,
                                tile_position=(0, 0), skip_group_check=True)
                            nc.tensor.matmul(
                                out=ps[E:128, psl], lhsT=w[E:128, wsl],
                                rhs=s[E:128, ssl], start=first, stop=True,
                                tile_position=(E, E), skip_group_check=True)
                        nc.scalar.activation(
                            out=h_new[:, i0 * E:(i0 + g) * E],
                            in_=ps[:, 0:g * E],
                            func=mybir.ActivationFunctionType.Relu)
                h_prev = h_new
                woff += n

            nc.sync.dma_start(out[:], h_prev[:, 0:E])

    nc.compile()
    return nc


def _get_nc():
    if "nc" not in _CACHE:
        _CACHE["nc"] = _build_nc()
    return _CACHE["nc"]


def _pack_core(c, node_ids, emb, bias_table):
    """Packed wt/lf/bi streams for core c (half-trees rooted at heap nodes
    15+2c and 16+2c)."""
    wt = np.empty((2, E, WT_SLOTS, E), dtype=np.float32)   # [half, k, slot, m]
    bi = np.empty((2, WT_SLOTS, E), dtype=np.float32)
    lfp = np.empty((2, E, LF_SLOTS, E), dtype=np.float32)  # [half, p, slot, m]
    for q in range(2):
        g0 = 15 + 2 * c + q
        woff = 0
        for lvl in range(HT_D - 2, -1, -1):
            n = 2 ** lvl
            start = (g0 + 1) * n - 1
            ids = node_ids[start:start + n]
            block = emb[ids].reshape(n, E, E)
            wt[q, :, woff:woff + n, :] = block.transpose(2, 0, 1)  # W^T strips
            bi[q, woff:woff + n, :] = bias_table[ids]
            woff += n
        start = (g0 + 1) * HT_LEAF - 1
        ids = node_ids[start:start + HT_LEAF]
        lfp[q] = emb[ids].reshape(HT_LEAF, E, E).transpose(1, 0, 2)
    return {
        "wt": np.ascontiguousarray(wt.reshape(128, WT_SLOTS * E)),
        "lf": np.ascontiguousarray(lfp.reshape(128, LF_SLOTS * E)),
        "bi": np.ascontiguousarray(bi.reshape(2, WT_SLOTS * E)),
    }


def kernel(node_ids, label, embedding, bias_table, proj_w, proj_b):
    from concourse.bass_utils import run_bass_kernel_spmd

    node_ids = np.asarray(node_ids).astype(np.int64)
    emb = np.ascontiguousarray(np.asarray(embedding, dtype=np.float32))
    bias_table = np.ascontiguousarray(np.asarray(bias_table, dtype=np.float32))
    proj_w = np.asarray(proj_w, dtype=np.float32)
    proj_b = np.asarray(proj_b, dtype=np.float32)
    label_i = int(np.asarray(label))

    nc = _get_nc()
    in_maps = [_pack_core(c, node_ids, emb, bias_table) for c in range(NCORES)]
    res = run_bass_kernel_spmd(nc, in_maps, core_ids=list(range(NCORES)))

    # top 4 levels (heap nodes 14..0) on host, float32 to match device math
    h = {}
    for c in range(NCORES):
        o = res.results[c]["out"].astype(np.float32)
        h[15 + 2 * c] = o[0:E]
        h[16 + 2 * c] = o[E:128]
    for g in range(14, -1, -1):
        s = h[2 * g + 1] + h[2 * g + 2]
        W = emb[node_ids[g]].reshape(E, E)
        b = bias_table[node_ids[g]]
        h[g] = np.maximum(W @ s + b[None, :], 0.0)

    root = h[0].reshape(-1)
    logits = root @ proj_w.T + proj_b
    m = logits.max()
    lse = m + np.log(np.exp(logits - m).sum())
    log_softmax = logits - lse
    loss = np.float32(-log_softmax[label_i])
    prediction = np.int64(np.argmax(logits))
    return prediction, loss


# revision 16
# speedup vs baseline: 1.3386x; 1.3386x over previous
"""Trainium2 Bass kernel for nn_AdditiveRecursiveNN (depth-13 binary tree of
64x64 matmuls with per-node weights gathered from a 50000x4096 table).

Sharding: data-parallel over the 16 independent depth-9 subtrees rooted at
heap nodes 15..30 -- TWO subtrees per NeuronCore, one per SBUF partition
half. The host packs each core's 1022 node weight matrices into a dense,
matmul-ready stream (pre-transposed W^T strips); each core runs both of its
subtrees bottom-up: h = relu(W @ (h_l + h_r) + b). The top 4 levels (15
nodes) are combined on the host.

Layout: "L" subtree lives in partitions 0:64, "R" in 64:128, node i of a
level at 64-column slot i. Everything is partition-aligned:
 - the pairwise child add is ONE full-width strided DVE op per level,
 - L/R matmuls use DIAGONAL PE quadrants (tile_position (0,0)/(64,64)),
   which are column-disjoint and run concurrently (same-column independent
   quadrant pairs like (0,0)+(64,0) fault the PE),
 - per-node bias is accumulated into PSUM first via K=1 ones-outer-product
   matmuls, so the PSUM drain is a single fused relu+bias on ScalarE.
"""
import sys
sys.path.insert(0, '/opt/trn_rl_repo')

import numpy as np
import ml_dtypes

E = 64
D = 13
N_NODES = 2 ** D - 1          # 8191
NCORES = 8
HT_D = 9                      # half-tree depth: levels 0..8
HT_LEAF = 2 ** (HT_D - 1)     # 256 leaves per half-tree
HT_INT = HT_LEAF - 1          # 255 internal nodes per half-tree
WT_SLOTS = HT_INT             # 255 (shared by wt and bi streams)
LF_SLOTS = HT_LEAF            # 256

_CACHE = {}
BIAS_MODE = "dve"


def _build_nc():
    import concourse.bacc as bacc
    import concourse.tile as tile
    import concourse.mybir as mybir

    f32 = mybir.dt.float32
    bf16 = mybir.dt.bfloat16
    nc = bacc.Bacc(None, target_bir_lowering=False)

    wt = nc.dram_tensor("wt", [128, WT_SLOTS * E], f32, kind="ExternalInput")
    lf = nc.dram_tensor("lf", [128, LF_SLOTS * E], f32, kind="ExternalInput")
    bi = nc.dram_tensor("bi", [2, WT_SLOTS * E], bf16, kind="ExternalInput")
    out = nc.dram_tensor("out", [128, E], f32, kind="ExternalOutput")

    with tile.TileContext(nc) as tc:
        with (
            tc.tile_pool(name="leaf", bufs=1) as pool_leaf,
            tc.tile_pool(name="h", bufs=2) as pool_h,
            tc.tile_pool(name="s", bufs=1) as pool_s,
            tc.tile_pool(name="wtp", bufs=2) as pool_wt,
            tc.tile_pool(name="btp", bufs=1) as pool_bt,
            tc.tile_pool(name="ps", bufs=8, space="PSUM") as pool_ps,
            tc.tile_pool(name="cst", bufs=1) as pool_c,
        ):
            ones = pool_c.tile([128, E], bf16)
            nc.gpsimd.memset(ones[:], 1.0)
            # ---- leaves: stream in, relu in place -> h8 ----
            h_prev = pool_leaf.tile([128, LF_SLOTS * E], f32)
            LCH = 32  # slots per chunk
            for t in range(0, LF_SLOTS, LCH):
                sl = slice(t * E, (t + LCH) * E)
                nc.sync.dma_start(h_prev[:, sl], lf[:, sl])
                nc.scalar.activation(h_prev[:, sl], h_prev[:, sl],
                                     func=mybir.ActivationFunctionType.Relu)

            woff = 0  # slot offset into wt/bi streams
            for lvl in range(HT_D - 2, -1, -1):
                n = 2 ** lvl  # nodes per half-tree this level
                # ---- s_i = h_2i + h_2i+1 : one strided DVE op ----
                s = pool_s.tile([128, n * E], f32, tag="s")
                pairs = h_prev[:, 0:2 * n * E].rearrange(
                    "p (t c) -> p t c", c=2 * E)
                nc.vector.tensor_add(
                    s[:].rearrange("p (t m) -> p t m", m=E),
                    pairs[:, :, 0:E], pairs[:, :, E:2 * E])

                h_new = pool_h.tile([128, n * E], f32, tag="h")

                CH = 32  # slots per weight chunk
                for t0 in range(0, n, CH):
                    tn = min(CH, n - t0)
                    w = pool_wt.tile([128, CH * E], f32, tag="wt")
                    nc.sync.dma_start(
                        w[:, 0:tn * E],
                        wt[:, (woff + t0) * E:(woff + t0 + tn) * E])
                    bt = pool_bt.tile([128, CH * E], bf16, tag="bt")
                    boff = woff + t0
                    nc.sync.dma_start(
                        bt[0:1, 0:tn * E], bi[0:1, boff * E:(boff + tn) * E])
                    nc.sync.dma_start(
                        bt[E:E + 1, 0:tn * E],
                        bi[1:2, boff * E:(boff + tn) * E])
                    for i0 in range(t0, t0 + tn, 8):
                        g = min(8, t0 + tn - i0)
                        ps = pool_ps.tile([128, 8 * E], f32, tag="ps")
                        first = False
                        bsl = slice((i0 - t0) * E, (i0 - t0 + g) * E)
                        nc.tensor.matmul(
                            out=ps[0:E, 0:g * E], lhsT=ones[0:1, :],
                            rhs=bt[0:1, bsl], start=True, stop=False,
                            tile_position=(0, 0), skip_group_check=True)
                        nc.tensor.matmul(
                            out=ps[E:128, 0:g * E], lhsT=ones[E:E + 1, :],
                            rhs=bt[E:E + 1, bsl], start=True, stop=False,
                            tile_position=(E, E), skip_group_check=True)
                        for i in range(i0, i0 + g):
                            psl = slice((i - i0) * E, (i - i0 + 1) * E)
                            wsl = slice((i - t0) * E, (i - t0 + 1) * E)
                            ssl = slice(i * E, (i + 1) * E)
                            nc.tensor.matmul(
                                out=ps[0:E, psl],
                                lhsT=w[0:E, wsl],
                                rhs=s[0:E, ssl],
                                start=first, stop=True,
                                tile_position=(0, 0), skip_group_check=True)
                            nc.tensor.matmul(
                                out=ps[E:128, psl],
                                lhsT=w[E:128, wsl],
                                rhs=s[E:128, ssl],
                                start=first, stop=True,
                                tile_position=(E, E), skip_group_check=True)
                        nc.scalar.activation(
                            out=h_new[:, i0 * E:(i0 + g) * E],
                            in_=ps[:, 0:g * E],
                            func=mybir.ActivationFunctionType.Relu)
                h_prev = h_new
                woff += n

            nc.sync.dma_start(out[:], h_prev[:, 0:E])

    nc.compile()
    return nc


def _get_nc():
    if "nc" not in _CACHE:
        _CACHE["nc"] = _build_nc()
    return _CACHE["nc"]


def _pack_core(c, node_ids, emb, bias_table):
    """Packed wt/lf/bi streams for core c (half-trees rooted at heap nodes
    15+2c and 16+2c)."""
    wt = np.empty((2, E, WT_SLOTS, E), dtype=np.float32)   # [half, k, slot, m]
    bi = np.empty((2, WT_SLOTS, E), dtype=np.float32)
    lfp = np.empty((2, E, LF_SLOTS, E), dtype=np.float32)  # [half, p, slot, m]
    for q in range(2):
        g0 = 15 + 2 * c + q
        woff = 0
        for lvl in range(HT_D - 2, -1, -1):
            n = 2 ** lvl
            start = (g0 + 1) * n - 1
            ids = node_ids[start:start + n]
            block = emb[ids].reshape(n, E, E)
            wt[q, :, woff:woff + n, :] = block.transpose(2, 0, 1)  # W^T strips
            bi[q, woff:woff + n, :] = bias_table[ids]
            woff += n
        start = (g0 + 1) * HT_LEAF - 1
        ids = node_ids[start:start + HT_LEAF]
        lfp[q] = emb[ids].reshape(HT_LEAF, E, E).transpose(1, 0, 2)
    return {
        "wt": np.ascontiguousarray(wt.reshape(128, WT_SLOTS * E)),
        "lf": np.ascontiguousarray(lfp.reshape(128, LF_SLOTS * E)),
        "bi": np.ascontiguousarray(bi.reshape(2, WT_SLOTS * E)).astype(ml_dtypes.bfloat16),
    }


def kernel(node_ids, label, embedding, bias_table, proj_w, proj_b):
    from concourse.bass_utils import run_bass_kernel_spmd

    node_ids = np.asarray(node_ids).astype(np.int64)
    emb = np.ascontiguousarray(np.asarray(embedding, dtype=np.float32))
    bias_table = np.ascontiguousarray(np.asarray(bias_table, dtype=np.float32))
    proj_w = np.asarray(proj_w, dtype=np.float32)
    proj_b = np.asarray(proj_b, dtype=np.float32)
    label_i = int(np.asarray(label))

    nc = _get_nc()
    in_maps = [_pack_core(c, node_ids, emb, bias_table) for c in range(NCORES)]
    res = run_bass_kernel_spmd(nc, in_maps, core_ids=list(range(NCORES)))

    # top 4 levels (heap nodes 14..0) on host, float32 to match device math
    h = {}
    for c in range(NCORES):
        o = res.results[c]["out"].astype(np.float32)
        h[15 + 2 * c] = o[0:E]
        h[16 + 2 * c] = o[E:128]
    for g in range(14, -1, -1):
        s = h[2 * g + 1] + h[2 * g + 2]
        W = emb[node_ids[g]].reshape(E, E)
        b = bias_table[node_ids[g]]
        h[g] = np.maximum(W @ s + b[None, :], 0.0)

    root = h[0].reshape(-1)
    logits = root @ proj_w.T + proj_b
    m = logits.max()
    lse = m + np.log(np.exp(logits - m).sum())
    log_softmax = logits - lse
    loss = np.float32(-log_softmax[label_i])
    prediction = np.int64(np.argmax(logits))
    return prediction, loss


# revision 17
# speedup vs baseline: 1.4200x; 1.0608x over previous
"""Trainium2 Bass kernel for nn_AdditiveRecursiveNN (depth-13 binary tree of
64x64 matmuls with per-node weights gathered from a 50000x4096 table).

Sharding: data-parallel over the 16 independent depth-9 subtrees rooted at
heap nodes 15..30 -- TWO subtrees per NeuronCore, one per SBUF partition
half. The host packs each core's 1022 node weight matrices into a dense,
matmul-ready stream (pre-transposed W^T strips); each core runs both of its
subtrees bottom-up: h = relu(W @ (h_l + h_r) + b). The top 4 levels (15
nodes) are combined on the host.

Layout: "L" subtree lives in partitions 0:64, "R" in 64:128, node i of a
level at 64-column slot i. Everything is partition-aligned:
 - the pairwise child add is ONE full-width strided DVE op per level,
 - L/R matmuls use DIAGONAL PE quadrants (tile_position (0,0)/(64,64)),
   which are column-disjoint and run concurrently (same-column independent
   quadrant pairs like (0,0)+(64,0) fault the PE),
 - per-node bias is accumulated into PSUM first via K=1 ones-outer-product
   matmuls, so the PSUM drain is a single fused relu+bias on ScalarE.
"""
import sys
sys.path.insert(0, '/opt/trn_rl_repo')

import numpy as np
import ml_dtypes

E = 64
D = 13
N_NODES = 2 ** D - 1          # 8191
NCORES = 8
HT_D = 9                      # half-tree depth: levels 0..8
HT_LEAF = 2 ** (HT_D - 1)     # 256 leaves per half-tree
HT_INT = HT_LEAF - 1          # 255 internal nodes per half-tree
WT_SLOTS = HT_INT             # 255 (shared by wt and bi streams)
LF_SLOTS = HT_LEAF            # 256

_CACHE = {}
BIAS_MODE = "dve"


def _build_nc():
    import concourse.bacc as bacc
    import concourse.tile as tile
    import concourse.mybir as mybir

    f32 = mybir.dt.float32
    bf16 = mybir.dt.bfloat16
    nc = bacc.Bacc(None, target_bir_lowering=False)

    wt = nc.dram_tensor("wt", [128, WT_SLOTS * E], f32, kind="ExternalInput")
    lf = nc.dram_tensor("lf", [128, LF_SLOTS * E], f32, kind="ExternalInput")
    bi = nc.dram_tensor("bi", [2, WT_SLOTS * E], bf16, kind="ExternalInput")
    out = nc.dram_tensor("out", [128, E], f32, kind="ExternalOutput")

    with tile.TileContext(nc) as tc:
        with (
            tc.tile_pool(name="leaf", bufs=1) as pool_leaf,
            tc.tile_pool(name="h", bufs=2) as pool_h,
            tc.tile_pool(name="s", bufs=1) as pool_s,
            tc.tile_pool(name="wtp", bufs=2) as pool_wt,
            tc.tile_pool(name="btp", bufs=2) as pool_bt,
            tc.tile_pool(name="ps", bufs=8, space="PSUM") as pool_ps,
            tc.tile_pool(name="cst", bufs=1) as pool_c,
        ):
            ones = pool_c.tile([128, E], bf16)
            nc.gpsimd.memset(ones[:], 1.0)
            # ---- leaves: stream in, relu in place -> h8 ----
            h_prev = pool_leaf.tile([128, LF_SLOTS * E], f32)
            LCH = 32  # slots per chunk
            for t in range(0, LF_SLOTS, LCH):
                sl = slice(t * E, (t + LCH) * E)
                nc.sync.dma_start(h_prev[:, sl], lf[:, sl])
                nc.scalar.activation(h_prev[:, sl], h_prev[:, sl],
                                     func=mybir.ActivationFunctionType.Relu)

            woff = 0  # slot offset into wt/bi streams
            for lvl in range(HT_D - 2, -1, -1):
                n = 2 ** lvl  # nodes per half-tree this level
                s = pool_s.tile([128, n * E], f32, tag="s")
                h_new = pool_h.tile([128, n * E], f32, tag="h")

                CH = 32  # slots per weight chunk
                for t0 in range(0, n, CH):
                    tn = min(CH, n - t0)
                    # ---- s_i = h_2i + h_2i+1 for this chunk: strided DVE op
                    pairs = h_prev[:, 2 * t0 * E:2 * (t0 + tn) * E].rearrange(
                        "p (t c) -> p t c", c=2 * E)
                    nc.vector.tensor_add(
                        s[:, t0 * E:(t0 + tn) * E].rearrange(
                            "p (t m) -> p t m", m=E),
                        pairs[:, :, 0:E], pairs[:, :, E:2 * E])
                    w = pool_wt.tile([128, CH * E], f32, tag="wt")
                    nc.sync.dma_start(
                        w[:, 0:tn * E],
                        wt[:, (woff + t0) * E:(woff + t0 + tn) * E])
                    bt = pool_bt.tile([128, CH * E], bf16, tag="bt")
                    boff = woff + t0
                    nc.sync.dma_start(
                        bt[0:1, 0:tn * E], bi[0:1, boff * E:(boff + tn) * E])
                    nc.sync.dma_start(
                        bt[E:E + 1, 0:tn * E],
                        bi[1:2, boff * E:(boff + tn) * E])
                    for i0 in range(t0, t0 + tn, 8):
                        g = min(8, t0 + tn - i0)
                        ps = pool_ps.tile([128, 8 * E], f32, tag="ps")
                        first = False
                        bsl = slice((i0 - t0) * E, (i0 - t0 + g) * E)
                        nc.tensor.matmul(
                            out=ps[0:E, 0:g * E], lhsT=ones[0:1, :],
                            rhs=bt[0:1, bsl], start=True, stop=False,
                            tile_position=(0, 0), skip_group_check=True)
                        nc.tensor.matmul(
                            out=ps[E:128, 0:g * E], lhsT=ones[E:E + 1, :],
                            rhs=bt[E:E + 1, bsl], start=True, stop=False,
                            tile_position=(E, E), skip_group_check=True)
                        for i in range(i0, i0 + g):
                            psl = slice((i - i0) * E, (i - i0 + 1) * E)
                            wsl = slice((i - t0) * E, (i - t0 + 1) * E)
                            ssl = slice(i * E, (i + 1) * E)
                            nc.tensor.matmul(
                                out=ps[0:E, psl],
                                lhsT=w[0:E, wsl],
                                rhs=s[0:E, ssl],
                                start=first, stop=True,
                                tile_position=(0, 0), skip_group_check=True)
                            nc.tensor.matmul(
                                out=ps[E:128, psl],
                                lhsT=w[E:128, wsl],
                                rhs=s[E:128, ssl],
                                start=first, stop=True,
                                tile_position=(E, E), skip_group_check=True)
                        nc.scalar.activation(
                            out=h_new[:, i0 * E:(i0 + g) * E],
                            in_=ps[:, 0:g * E],
                            func=mybir.ActivationFunctionType.Relu)
                h_prev = h_new
                woff += n

            nc.sync.dma_start(out[:], h_prev[:, 0:E])

    nc.compile()
    return nc


def _get_nc():
    if "nc" not in _CACHE:
        _CACHE["nc"] = _build_nc()
    return _CACHE["nc"]


def _pack_core(c, node_ids, emb, bias_table):
    """Packed wt/lf/bi streams for core c (half-trees rooted at heap nodes
    15+2c and 16+2c)."""
    wt = np.empty((2, E, WT_SLOTS, E), dtype=np.float32)   # [half, k, slot, m]
    bi = np.empty((2, WT_SLOTS, E), dtype=np.float32)
    lfp = np.empty((2, E, LF_SLOTS, E), dtype=np.float32)  # [half, p, slot, m]
    for q in range(2):
        g0 = 15 + 2 * c + q
        woff = 0
        for lvl in range(HT_D - 2, -1, -1):
            n = 2 ** lvl
            start = (g0 + 1) * n - 1
            ids = node_ids[start:start + n]
            block = emb[ids].reshape(n, E, E)
            wt[q, :, woff:woff + n, :] = block.transpose(2, 0, 1)  # W^T strips
            bi[q, woff:woff + n, :] = bias_table[ids]
            woff += n
        start = (g0 + 1) * HT_LEAF - 1
        ids = node_ids[start:start + HT_LEAF]
        lfp[q] = emb[ids].reshape(HT_LEAF, E, E).transpose(1, 0, 2)
    return {
        "wt": np.ascontiguousarray(wt.reshape(128, WT_SLOTS * E)),
        "lf": np.ascontiguousarray(lfp.reshape(128, LF_SLOTS * E)),
        "bi": np.ascontiguousarray(bi.reshape(2, WT_SLOTS * E)).astype(ml_dtypes.bfloat16),
    }


def kernel(node_ids, label, embedding, bias_table, proj_w, proj_b):
    from concourse.bass_utils import run_bass_kernel_spmd

    node_ids = np.asarray(node_ids).astype(np.int64)
    emb = np.ascontiguousarray(np.asarray(embedding, dtype=np.float32))
    bias_table = np.ascontiguousarray(np.asarray(bias_table, dtype=np.float32))
    proj_w = np.asarray(proj_w, dtype=np.float32)
    proj_b = np.asarray(proj_b, dtype=np.float32)
    label_i = int(np.asarray(label))

    nc = _get_nc()
    in_maps = [_pack_core(c, node_ids, emb, bias_table) for c in range(NCORES)]
    res = run_bass_kernel_spmd(nc, in_maps, core_ids=list(range(NCORES)))

    # top 4 levels (heap nodes 14..0) on host, float32 to match device math
    h = {}
    for c in range(NCORES):
        o = res.results[c]["out"].astype(np.float32)
        h[15 + 2 * c] = o[0:E]
        h[16 + 2 * c] = o[E:128]
    for g in range(14, -1, -1):
        s = h[2 * g + 1] + h[2 * g + 2]
        W = emb[node_ids[g]].reshape(E, E)
        b = bias_table[node_ids[g]]
        h[g] = np.maximum(W @ s + b[None, :], 0.0)

    root = h[0].reshape(-1)
    logits = root @ proj_w.T + proj_b
    m = logits.max()
    lse = m + np.log(np.exp(logits - m).sum())
    log_softmax = logits - lse
    loss = np.float32(-log_softmax[label_i])
    prediction = np.int64(np.argmax(logits))
    return prediction, loss
